# revision 5
# baseline (speedup 1.0000x reference)
"""MllamaTextCrossAttention kernel for 8 Trainium2 NeuronCores.

Strategy: tensor-parallel over heads (4 q-heads + 1 kv-head per core).
Each core computes q/k/v projections for its shard, fused QK-RMS-norm,
block-diagonal varlen attention (segments baked in at build time from the
actual cu_seqlen values), and a row-parallel o_proj partial of the full
[1024, 4096] output. The host sums the 8 partials.

Layout: cross_attention_states is repacked with PER-SEGMENT padding so every
segment spans a whole number of 128-kpos tiles; pad positions are masked in
the softmax via a per-partition bias of -50 on the exp activation (exp -> 0),
so no boundary patch DMAs are needed.  All attention runs in transposed
layout (features on partitions): qT [d, tok], kT [d, kpos], scoresT
[kpos, q], attnT [d, q].  Softmax denominators use a DVE accumulation tree +
GpSimd partition_all_reduce (no PE work, no PSUM).  Attention units and
o_proj chains are drip-fed between KV-projection matmul groups so the PE
queue never drains (keeps HAM at 8/8).
"""
import os
import sys

if "/opt/trn_rl_repo" not in sys.path:
    sys.path.insert(0, "/opt/trn_rl_repo")

import numpy as np

HIDDEN = 4096
N_HEADS = 32
N_KV = 8
HD = 128
EPS = 1e-5
SCALE = HD ** -0.5
TQ = 1024
TK = 6404
NCORES = 8
HPC = N_HEADS // NCORES  # 4 q-heads per core
P = 128
NC = HIDDEN // P         # 32 contraction chunks


def _segments(cu_q, cu_k):
    eq = [0] + [int(min(max(int(v), 0), TQ)) for v in cu_q] + [TQ]
    ek = [0] + [int(min(max(int(v), 0), TK)) for v in cu_k] + [TK]
    segs = []
    for i in range(len(eq) - 1):
        q0, q1 = eq[i], eq[i + 1]
        k0, k1 = ek[i], ek[i + 1]
        if q1 <= q0:
            continue
        if k1 <= k0:
            segs.append(dict(q0=q0, q1=q1, k0=0, k1=TK, special=True))
        else:
            segs.append(dict(q0=q0, q1=q1, k0=k0, k1=k1, special=False))
    t = 0
    for i, sg in enumerate(segs):
        sg["i"] = i
        sg["t0"] = t
        sg["nt"] = (sg["k1"] - sg["k0"] + P - 1) // P
        sg["rem"] = (sg["k1"] - sg["k0"]) % P  # real rows in last tile (0=full)
        t += sg["nt"]
    return segs, t


def _build(segs, TT):
    import concourse.bass as bass  # noqa: F401
    import concourse.tile as tile
    from concourse import bacc, mybir
    from concourse.bass_isa import ReduceOp

    F32 = mybir.dt.float32
    F16 = mybir.dt.float16
    AF = mybir.ActivationFunctionType
    MUL = mybir.AluOpType.mult
    ADD = mybir.AluOpType.add

    SLABS = (TT + 3) // 4          # 512-kpos slabs over the padded stream
    NTI = SLABS * 4                # 128-kpos tiles (incl. tail pad tiles)
    NSEG = len(segs)

    nc = bacc.Bacc("TRN2", target_bir_lowering=False, debug=False,
                   num_devices=NCORES)

    hT = nc.declare_dram_parameter("hT", [P, 2, NC, 512], F16, isOutput=False)
    cT = nc.declare_dram_parameter("cT", [P, SLABS, NC, 512], F16,
                                   isOutput=False)
    wqT = nc.declare_dram_parameter("wqT", [P, NC, P * HPC], F16,
                                    isOutput=False)
    wkv = nc.declare_dram_parameter("wkv", [P, NC, 2 * P], F16, isOutput=False)
    woT = nc.declare_dram_parameter("woT", [P, 8, HPC, 512], F16,
                                    isOutput=False)
    wqk = nc.declare_dram_parameter("wqk", [1, P], F32, isOutput=False)
    onec = nc.declare_dram_parameter("onec", [P, 1], F16, isOutput=False)
    onec2 = nc.declare_dram_parameter("onec2", [P, 2], F16, isOutput=False)
    identd = nc.declare_dram_parameter("identd", [P, P], F16, isOutput=False)
    biasd = nc.declare_dram_parameter("biasd", [P, NSEG], F32, isOutput=False)
    zerosd = nc.declare_dram_parameter("zerosd", [P, 512], F16, isOutput=False)
    onesd = nc.declare_dram_parameter("onesd", [P, 512], F16, isOutput=False)
    out = nc.declare_dram_parameter("o", [TQ, HIDDEN], F32, isOutput=True)

    any_special = any(sg["special"] for sg in segs)

    with tile.TileContext(nc) as tc:
        with tc.tile_pool(name="persist", bufs=1) as pp:
            qT = pp.tile([P, HPC, TQ + 2], F16)     # q transposed (+2 pad)
            kT = pp.tile([P, NTI, P], F16)          # k transposed, 128-blocks
            vN = pp.tile([P, NTI, P], F16)          # v natural, 128-blocks
            aT = pp.tile([P, HPC, TQ], F16)         # attn output transposed
            A_sb = pp.tile([P, NTI], F32)           # 1/rms_k per kpos tile
            wo_sb = pp.tile([P, 8, HPC, 512], F16)  # o_proj weights, resident
            onec_sb = pp.tile([P, 1], F16)
            onec2_sb = pp.tile([P, 2], F16)
            wqk_sb = pp.tile([1, P], F32)
            ident_sb = pp.tile([P, P], F16)
            bias_sb = pp.tile([P, NSEG], F32)
            eps_sb = pp.tile([P, 1], F32)
            nc.vector.memset(eps_sb[:], EPS)
            nc.sync.dma_start(onec_sb[:], onec[:])
            nc.sync.dma_start(onec2_sb[:], onec2[:])
            nc.sync.dma_start(wqk_sb[:], wqk[:])
            nc.sync.dma_start(ident_sb[:], identd[:])
            nc.sync.dma_start(bias_sb[:], biasd[:])

            # ---- Phase Q: q projection + fused RMS norm --------------------
            with tc.tile_pool(name="qp", bufs=2) as qp, \
                 tc.tile_pool(name="qps", bufs=1, space="PSUM") as qps, \
                 tc.tile_pool(name="qps2", bufs=1, space="PSUM") as qps2:
                for half in range(2):
                    tsl = slice(half * 512, (half + 1) * 512)
                    psq = [qps.tile([P, 512], F32, tag=f"q{f}",
                                    name=f"psq{f}") for f in range(HPC)]
                    for qr in range(16):
                        wq_q = qp.tile([P, 2, 512], F16, tag="wqq", name="wqq")
                        nc.sync.dma_start(wq_q[:],
                                          wqT[:, qr * 2:(qr + 1) * 2, :])
                        ht_q = qp.tile([P, 2, 512], F16, tag="htq", name="htq")
                        nc.sync.dma_start(ht_q[:],
                                          hT[:, half, qr * 2:(qr + 1) * 2, :])
                        for cc in range(2):
                            for f in range(HPC):
                                nc.tensor.matmul(
                                    psq[f][:],
                                    wq_q[:, cc, f * P:(f + 1) * P],
                                    ht_q[:, cc, :],
                                    start=(qr == 0 and cc == 0),
                                    stop=(qr == 15 and cc == 1))
                    for f in range(HPC):
                        qsq = qp.tile([P, 512], F16, tag="qsq", name="qsq")
                        nc.vector.tensor_copy(qT[:, f, tsl], psq[f][:])
                        nc.vector.tensor_tensor(qsq[:], qT[:, f, tsl],
                                                qT[:, f, tsl], MUL)
                        pss = qps2.tile([1, 512], F32, tag="pss", name="pssq")
                        nc.tensor.matmul(pss[:], onec_sb[:], qsq[:],
                                         start=True, stop=True)
                        sq = qp.tile([1, 512], F32, tag="sq", name="sq")
                        nc.scalar.activation(sq[:], pss[:], AF.Sqrt,
                                             bias=eps_sb[0:1], scale=1.0 / HD)
                        nc.vector.reciprocal(sq[:], sq[:])
                        psb = qps2.tile([P, 512], F32, tag="psb", name="psbq")
                        nc.tensor.matmul(psb[:], wqk_sb[:], sq[:],
                                         start=True, stop=True)
                        nc.vector.tensor_tensor(qT[:, f, tsl], qT[:, f, tsl],
                                                psb[:], MUL)

            # ---- pools for KV / attention / o_proj -------------------------
            kvw_cm = tc.tile_pool(name="kvw", bufs=1)
            kvw = kvw_cm.__enter__()
            ctp_cm = tc.tile_pool(name="ctp", bufs=6)
            ctp = ctp_cm.__enter__()
            kvp_cm = tc.tile_pool(name="kvp", bufs=2)
            kvp = kvp_cm.__enter__()
            ep_cm = tc.tile_pool(name="ep", bufs=2)
            ep = ep_cm.__enter__()
            accp_cm = tc.tile_pool(name="accp", bufs=2)
            accp = accp_cm.__enter__()
            op_cm = tc.tile_pool(name="op", bufs=2)
            op = op_cm.__enter__()
            pssp_cm = tc.tile_pool(name="pssp", bufs=2, space="PSUM")
            pssp = pssp_cm.__enter__()
            psap_cm = tc.tile_pool(name="psap", bufs=1, space="PSUM")
            psap = psap_cm.__enter__()
            psro_cm = tc.tile_pool(name="psro", bufs=1, space="PSUM")
            psro = psro_cm.__enter__()
            kvps_cm = tc.tile_pool(name="kvps", bufs=1, space="PSUM")
            kvps = kvps_cm.__enter__()

            wkv_sb = kvw.tile([P, NC, 2 * P], F16)
            nc.sync.dma_start(wkv_sb[:], wkv[:])
            nc.sync.dma_start(wo_sb[:], woT[:])

            pso_pool = [psro]

            def unit_gen(sg, qc0, hp):
                t0, nt = sg["t0"], sg["nt"]
                qc1 = min(qc0 + 256, sg["q1"])
                nq = qc1 - qc0
                nqp = nq + (nq & 1)
                h0 = 2 * hp
                E = ep.tile([P, nt, 2, nqp], F16, tag="E", name="E")
                for ti in range(nt):
                    t = t0 + ti
                    pss = pssp.tile([P, 2, nqp], F32, tag="pss", name="pss")
                    nc.tensor.matmul(pss[:], kT[:, t, :],
                                     qT[:, h0:h0 + 2, qc0:qc0 + nqp],
                                     start=True, stop=True)
                    if sg["rem"] and ti == nt - 1:
                        nc.scalar.activation(E[:, ti, :, :], pss[:], AF.Exp,
                                             bias=bias_sb[:,
                                                          sg["i"]:sg["i"] + 1],
                                             scale=A_sb[:, t:t + 1])
                    else:
                        nc.scalar.activation(E[:, ti, :, :], pss[:], AF.Exp,
                                             scale=A_sb[:, t:t + 1])
                    if ti % 7 == 6:
                        yield
                yield
                psa = psap.tile([P, 2, nqp], F32, tag="psa", name="psa")
                for ti in range(nt):
                    nc.tensor.matmul(psa[:], vN[:, t0 + ti, :],
                                     E[:, ti, :, :],
                                     start=(ti == 0), stop=(ti == nt - 1))
                    if ti == nt // 2:
                        yield
                acc = accp.tile([P, 2, nqp], F16, tag="acc", name="acc")
                if nt == 1:
                    nc.vector.tensor_copy(acc[:], E[:, 0, :, :])
                else:
                    nc.vector.tensor_tensor(acc[:], E[:, 0, :, :],
                                            E[:, 1, :, :], ADD)
                    for ti in range(2, nt):
                        nc.vector.tensor_tensor(acc[:], acc[:],
                                                E[:, ti, :, :], ADD)
                den = accp.tile([P, 2, nqp], F32, tag="den", name="den")
                nc.gpsimd.partition_all_reduce(den[:], acc[:], P,
                                               ReduceOp.add)
                rec = accp.tile([P, 2, nqp], F32, tag="rec", name="rec")
                nc.vector.reciprocal(rec[:], den[:])
                for j in range(2):
                    nc.vector.tensor_tensor(aT[:, h0 + j, qc0:qc1],
                                            psa[:, j, 0:nq],
                                            rec[:, j, 0:nq], MUL)

            def unit_gen_special(sg, qc0, hp):
                # degenerate segment (empty kv): uniform attention over ALL
                # real kpos.  E = 1 on real rows, 0 on pad rows, spanning
                # every segment's tiles.
                qc1 = min(qc0 + 256, sg["q1"])
                nq = qc1 - qc0
                nqp = nq + (nq & 1)
                h0 = 2 * hp
                psa = psap.tile([P, 2, nqp], F32, tag="psa", name="psa")
                acc = accp.tile([P, 2, nqp], F16, tag="acc", name="acc")
                first = True
                tlist = []
                for s2 in segs:
                    for ti in range(s2["nt"]):
                        full = not (s2["rem"] and ti == s2["nt"] - 1)
                        tlist.append((s2["t0"] + ti,
                                      P if full else s2["rem"]))
                CH = 13
                for c0 in range(0, len(tlist), CH):
                    chunk = tlist[c0:c0 + CH]
                    E = ep.tile([P, len(chunk), 2, nqp], F16, tag="E",
                                name="E")
                    for j, (t, hi) in enumerate(chunk):
                        nc.sync.dma_start(E[0:hi, j, :, :],
                                          onesd[0:hi, :2 * nqp])
                        if hi < P:
                            nc.sync.dma_start(E[hi:P, j, :, :],
                                              zerosd[hi:P, :2 * nqp])
                    for j, (t, hi) in enumerate(chunk):
                        last = (c0 + CH >= len(tlist) and j == len(chunk) - 1)
                        nc.tensor.matmul(psa[:], vN[:, t, :], E[:, j, :, :],
                                         start=(c0 == 0 and j == 0),
                                         stop=last)
                    for j in range(len(chunk)):
                        if c0 == 0 and j == 0:
                            nc.vector.tensor_copy(acc[:], E[:, 0, :, :])
                        else:
                            nc.vector.tensor_tensor(acc[:], acc[:],
                                                    E[:, j, :, :], ADD)
                    yield
                den = accp.tile([P, 2, nqp], F32, tag="den", name="den")
                nc.gpsimd.partition_all_reduce(den[:], acc[:], P,
                                               ReduceOp.add)
                rec = accp.tile([P, 2, nqp], F32, tag="rec", name="rec")
                nc.vector.reciprocal(rec[:], den[:])
                for j in range(2):
                    nc.vector.tensor_tensor(aT[:, h0 + j, qc0:qc1],
                                            psa[:, j, 0:nq],
                                            rec[:, j, 0:nq], MUL)

            def oproj_gen(qt):
                for n8 in range(8):
                    pso = pso_pool[0].tile([P, 512], F32, tag="pso",
                                           name="pso")
                    for co in range(HPC):
                        nc.tensor.matmul(pso[:],
                                         aT[:, co, qt * P:(qt + 1) * P],
                                         wo_sb[:, n8, co, :],
                                         start=(co == 0),
                                         stop=(co == HPC - 1))
                    osb = op.tile([P, 512], F32, tag="osb", name="osb")
                    nc.vector.tensor_copy(osb[:], pso[:])
                    nc.sync.dma_start(
                        out[qt * P:(qt + 1) * P, n8 * 512:(n8 + 1) * 512],
                        osb[:])
                    if n8 < 7:
                        yield

            # bookkeeping
            NQT = TQ // P
            qt_segs = [set() for _ in range(NQT)]
            for sg in segs:
                for qt in range(sg["q0"] // P, (sg["q1"] + P - 1) // P):
                    qt_segs[qt].add(sg["i"])
            unit_total = {sg["i"]: 0 for sg in segs}
            unit_done = {sg["i"]: 0 for sg in segs}
            for sg in segs:
                for _ in range(sg["q0"], sg["q1"], 256):
                    unit_total[sg["i"]] += HPC // 2
            o_enq = [False] * NQT
            aq = []  # [(seg_i, generator)]
            oq = []  # [generator]

            def finish_unit(si):
                unit_done[si] += 1
                for qt in range(NQT):
                    if o_enq[qt]:
                        continue
                    if all(unit_done[s2] == unit_total[s2]
                           for s2 in qt_segs[qt]):
                        o_enq[qt] = True
                        oq.append(oproj_gen(qt))

            def advance_attn():
                si, g = aq[0]
                try:
                    next(g)
                except StopIteration:
                    aq.pop(0)
                    finish_unit(si)

            def advance_oproj():
                g = oq[0]
                try:
                    next(g)
                except StopIteration:
                    oq.pop(0)

            def drip():
                if aq:
                    advance_attn()
                elif oq:
                    advance_oproj()

            def enqueue_ready(kmax_t):
                for sg in segs:
                    if sg.get("enq") or sg["special"]:
                        continue
                    if sg["t0"] + sg["nt"] <= kmax_t:
                        sg["enq"] = True
                        for qc0 in range(sg["q0"], sg["q1"], 256):
                            for hp in range(HPC // 2):
                                aq.append((sg["i"], unit_gen(sg, qc0, hp)))

            # ---- KV slabs with drip-fed attention/o_proj -------------------
            for s in range(SLABS):
                psk = kvps.tile([P, 512], F32, tag="psk", name="psk")
                psv = kvps.tile([P, 512], F32, tag="psv", name="psv")
                for q4 in range(8):
                    ct_q = ctp.tile([P, 4, 512], F16, tag="ctq", name="ctq")
                    nc.sync.dma_start(ct_q[:],
                                      cT[:, s, q4 * 4:(q4 + 1) * 4, :])
                    for cc in range(4):
                        c = q4 * 4 + cc
                        nc.tensor.matmul(psk[:], wkv_sb[:, c, 0:P],
                                         ct_q[:, cc, :],
                                         start=(c == 0), stop=(c == NC - 1))
                        nc.tensor.matmul(psv[:], wkv_sb[:, c, P:2 * P],
                                         ct_q[:, cc, :],
                                         start=(c == 0), stop=(c == NC - 1))
                    drip()
                    if len(aq) + len(oq) > 6:
                        drip()
                # slab drain: k copy + rms stats + v transposes
                nc.vector.tensor_copy(kT[:, 4 * s:4 * s + 4, :], psk[:])
                ksq = kvp.tile([P, 512], F16, tag="ksq", name="ksq")
                kslab = kT[:, 4 * s:4 * s + 4, :]
                nc.vector.tensor_tensor(ksq[:], kslab, kslab, MUL)
                vstage = kvp.tile([P, 512], F16, tag="vstage", name="vstage")
                nc.vector.tensor_copy(vstage[:], psv[:])
                sqk4 = kvp.tile([P, 4], F32, tag="sqk4", name="sqk4")
                for t in range(4):
                    psr = psro.tile([P, 2], F32, tag="psr", name="psr")
                    nc.tensor.matmul(psr[:], ksq[:, t * P:(t + 1) * P],
                                     onec2_sb[:], start=True, stop=True)
                    nc.vector.tensor_copy(sqk4[:, t:t + 1], psr[:, 0:1])
                    pst = kvps.tile([P, P], F16, tag="pst", name="pst")
                    nc.tensor.transpose(pst[:], vstage[:, t * P:(t + 1) * P],
                                        ident_sb[:])
                    nc.vector.tensor_copy(vN[:, 4 * s + t, :], pst[:])
                    if t % 2 == 1:
                        drip()
                sqk4b = kvp.tile([P, 4], F32, tag="sqk4b", name="sqk4b")
                nc.scalar.activation(sqk4b[:], sqk4[:], AF.Sqrt,
                                     bias=eps_sb[:], scale=1.0 / HD)
                nc.vector.reciprocal(A_sb[:, 4 * s:4 * s + 4], sqk4b[:])
                enqueue_ready(4 * (s + 1))

            # ---- tail: close KV psum, widen o_proj psum --------------------
            kvps_cm.__exit__(None, None, None)
            psro_cm.__exit__(None, None, None)
            pso2_cm = tc.tile_pool(name="pso2", bufs=2, space="PSUM")
            pso2 = pso2_cm.__enter__()
            pso_pool[0] = pso2

            if any_special:
                for sg in segs:
                    if sg["special"] and not sg.get("enq"):
                        sg["enq"] = True
                        for qc0 in range(sg["q0"], sg["q1"], 256):
                            for hp in range(HPC // 2):
                                aq.append((sg["i"],
                                           unit_gen_special(sg, qc0, hp)))

            while aq:
                si, g = aq.pop(0)
                try:
                    next(g)
                    aq.append((si, g))
                except StopIteration:
                    finish_unit(si)
                if oq:
                    advance_oproj()
            while oq:
                advance_oproj()

            assert all(o_enq), "o_proj bookkeeping failed"

            pso2_cm.__exit__(None, None, None)
            psap_cm.__exit__(None, None, None)
            pssp_cm.__exit__(None, None, None)
            op_cm.__exit__(None, None, None)
            accp_cm.__exit__(None, None, None)
            ep_cm.__exit__(None, None, None)
            kvp_cm.__exit__(None, None, None)
            ctp_cm.__exit__(None, None, None)
            kvw_cm.__exit__(None, None, None)

    nc.finalize()
    return nc, SLABS


def _prepare(inputs):
    gi = {k: np.asarray(v) for k, v in inputs.items()}
    hs = np.ascontiguousarray(gi["hidden_states"], dtype=np.float16)
    cs = np.ascontiguousarray(gi["cross_attention_states"], dtype=np.float16)
    Wq = np.ascontiguousarray(gi["Wq"], dtype=np.float16)
    Wk = np.ascontiguousarray(gi["Wk"], dtype=np.float16)
    Wv = np.ascontiguousarray(gi["Wv"], dtype=np.float16)
    Wo = np.ascontiguousarray(gi["Wo"], dtype=np.float16)
    qw = np.asarray(gi["q_norm_w"], dtype=np.float32).reshape(-1)
    kw = np.asarray(gi["k_norm_w"], dtype=np.float32).reshape(-1)
    cu_q = np.asarray(gi["cu_seqlen_q"]).reshape(-1)
    cu_k = np.asarray(gi["cu_seqlen_k"]).reshape(-1)

    segs, TT = _segments(cu_q, cu_k)
    nc, SLABS = _build(segs, TT)
    TKP = SLABS * 512

    # packed layouts: partition-major with long contiguous per-partition runs
    hTd = np.ascontiguousarray(
        hs.T.reshape(NC, P, 2, 512).transpose(1, 2, 0, 3))   # [128,2,32,512]
    # per-segment padded kpos stream
    cTp = np.zeros((HIDDEN, TKP), np.float16)
    for sg in segs:
        o = sg["t0"] * P
        cTp[:, o:o + sg["k1"] - sg["k0"]] = cs[sg["k0"]:sg["k1"]].T
    cTd = np.ascontiguousarray(
        cTp.reshape(NC, P, SLABS, 512).transpose(1, 2, 0, 3))
    wqkv = (qw * kw * SCALE).reshape(1, P).astype(np.float32)
    onec = np.ones((P, 1), np.float16)
    onec2 = np.ones((P, 2), np.float16)
    ident = np.eye(P, dtype=np.float16)
    biasv = np.zeros((P, len(segs)), np.float32)
    for sg in segs:
        if sg["rem"]:
            biasv[sg["rem"]:, sg["i"]] = -50.0
    zeros = np.zeros((P, 512), np.float16)
    ones = np.ones((P, 512), np.float16)

    in_maps = []
    for c in range(NCORES):
        fsl = slice(c * P * HPC, (c + 1) * P * HPC)
        ksl = slice(c * P, (c + 1) * P)
        wq_d = np.ascontiguousarray(
            Wq[fsl, :].T.reshape(NC, P, P * HPC).transpose(1, 0, 2))
        wkv_d = np.concatenate([
            Wk[ksl, :].T.reshape(NC, P, P).transpose(1, 0, 2),
            Wv[ksl, :].T.reshape(NC, P, P).transpose(1, 0, 2)], axis=2)
        wo_d = np.ascontiguousarray(
            Wo[:, fsl].T.reshape(HPC, P, 8, 512).transpose(1, 2, 0, 3))
        in_maps.append({
            "hT": hTd,
            "cT": cTd,
            "wqT": wq_d,
            "wkv": np.ascontiguousarray(wkv_d),
            "woT": wo_d,
            "wqk": wqkv,
            "onec": onec,
            "onec2": onec2,
            "identd": ident,
            "biasd": biasv,
            "zerosd": zeros,
            "onesd": ones,
        })

    return nc, in_maps


def _reduce(results) -> np.ndarray:
    o = np.zeros((TQ, HIDDEN), np.float64)
    for c in range(NCORES):
        o += results[c]["o"].astype(np.float64)
    return o.astype(np.float32)


def kernel(**inputs) -> np.ndarray:
    from concourse.bass_utils import run_bass_kernel_spmd

    nc, in_maps = _prepare(inputs)
    r = run_bass_kernel_spmd(nc, in_maps, list(range(NCORES)))
    return _reduce(r.results)


# revision 20
# speedup vs baseline: 1.1124x; 1.1124x over previous
"""MllamaTextCrossAttention kernel for 8 Trainium2 NeuronCores.

Strategy: tensor-parallel over heads (4 q-heads + 1 kv-head per core).
Each core computes q/k/v projections for its shard, fused QK-RMS-norm,
block-diagonal varlen attention (segments baked in at build time from the
actual cu_seqlen values), and a row-parallel o_proj partial of the full
[1024, 4096] output. The host sums the 8 partials.

Layout: cross_attention_states is repacked with PER-SEGMENT padding so every
segment spans a whole number of 128-kpos tiles; pad positions are masked in
the softmax via a per-partition bias of -50 on the exp activation (exp -> 0),
so no boundary patch DMAs are needed.  All attention runs in transposed
layout (features on partitions): qT [d, tok], kT [d, kpos], scoresT
[kpos, q], attnT [d, q].  Softmax denominators use a DVE accumulation tree +
GpSimd partition_all_reduce (no PE work, no PSUM).  Attention units and
o_proj chains are drip-fed between KV-projection matmul groups so the PE
queue never drains (keeps HAM at 8/8).
"""
import os
import sys

if "/opt/trn_rl_repo" not in sys.path:
    sys.path.insert(0, "/opt/trn_rl_repo")

import numpy as np

HIDDEN = 4096
N_HEADS = 32
N_KV = 8
HD = 128
EPS = 1e-5
SCALE = HD ** -0.5
TQ = 1024
TK = 6404
NCORES = 8
HPC = N_HEADS // NCORES  # 4 q-heads per core
P = 128
NC = HIDDEN // P         # 32 contraction chunks


def _segments(cu_q, cu_k):
    eq = [0] + [int(min(max(int(v), 0), TQ)) for v in cu_q] + [TQ]
    ek = [0] + [int(min(max(int(v), 0), TK)) for v in cu_k] + [TK]
    segs = []
    for i in range(len(eq) - 1):
        q0, q1 = eq[i], eq[i + 1]
        k0, k1 = ek[i], ek[i + 1]
        if q1 <= q0:
            continue
        if k1 <= k0:
            segs.append(dict(q0=q0, q1=q1, k0=0, k1=TK, special=True))
        else:
            segs.append(dict(q0=q0, q1=q1, k0=k0, k1=k1, special=False))
    t = 0
    for i, sg in enumerate(segs):
        sg["i"] = i
        sg["t0"] = t
        sg["nt"] = (sg["k1"] - sg["k0"] + P - 1) // P
        sg["rem"] = (sg["k1"] - sg["k0"]) % P  # real rows in last tile (0=full)
        t += sg["nt"]
    return segs, t


def _build(segs, TT):
    import concourse.bass as bass  # noqa: F401
    import concourse.tile as tile
    from concourse import bacc, mybir
    from concourse.bass_isa import ReduceOp

    F32 = mybir.dt.float32
    F16 = mybir.dt.float16
    AF = mybir.ActivationFunctionType
    MUL = mybir.AluOpType.mult
    ADD = mybir.AluOpType.add

    SLABS = (TT + 3) // 4          # 512-kpos slabs over the padded stream
    NTI = SLABS * 4                # 128-kpos tiles (incl. tail pad tiles)
    NSEG = len(segs)

    nc = bacc.Bacc("TRN2", target_bir_lowering=False, debug=False,
                   num_devices=NCORES)

    hT = nc.declare_dram_parameter("hT", [P, 2, NC, 512], F16, isOutput=False)
    cT = nc.declare_dram_parameter("cT", [P, SLABS, NC, 512], F16,
                                   isOutput=False)
    wqT = nc.declare_dram_parameter("wqT", [P, NC, P * HPC], F16,
                                    isOutput=False)
    wkv = nc.declare_dram_parameter("wkv", [P, NC, 2 * P], F16, isOutput=False)
    woT = nc.declare_dram_parameter("woT", [P, 8, HPC, 512], F16,
                                    isOutput=False)
    wqk = nc.declare_dram_parameter("wqk", [1, P], F32, isOutput=False)
    onec = nc.declare_dram_parameter("onec", [P, 1], F16, isOutput=False)
    onec2 = nc.declare_dram_parameter("onec2", [P, 2], F16, isOutput=False)
    oner = nc.declare_dram_parameter("oner", [1, P], F32, isOutput=False)
    identd = nc.declare_dram_parameter("identd", [P, P], F16, isOutput=False)
    biasd = nc.declare_dram_parameter("biasd", [P, NSEG], F32, isOutput=False)
    zerosd = nc.declare_dram_parameter("zerosd", [P, 512], F16, isOutput=False)
    onesd = nc.declare_dram_parameter("onesd", [P, 512], F16, isOutput=False)
    out = nc.declare_dram_parameter("o", [TQ, HIDDEN], F16, isOutput=True)

    any_special = any(sg["special"] for sg in segs)

    with tile.TileContext(nc) as tc:
        with tc.tile_pool(name="persist", bufs=1) as pp:
            qT = pp.tile([P, HPC, TQ + 2], F16)     # q transposed (+2 pad)
            kT = pp.tile([P, NTI, P], F16)          # k transposed, 128-blocks
            vN = pp.tile([P, NTI, P], F16)          # v natural, 128-blocks
            aT = pp.tile([P, HPC, TQ], F16)         # attn output transposed
            A_sb = pp.tile([P, NTI], F32)           # 1/rms_k per kpos tile
            wo_sb = pp.tile([P, 8, HPC, 512], F16)  # o_proj weights, resident
            onec_sb = pp.tile([P, 1], F16)
            onec2_sb = pp.tile([P, 2], F16)
            oner_sb = pp.tile([1, P], F32)
            wqk_sb = pp.tile([1, P], F32)
            ident_sb = pp.tile([P, P], F16)
            bias_sb = pp.tile([P, NSEG], F32)
            eps_sb = pp.tile([P, 1], F32)
            nc.vector.memset(eps_sb[:], EPS)
            nc.sync.dma_start(onec_sb[:], onec[:])
            nc.sync.dma_start(onec2_sb[:], onec2[:])
            nc.sync.dma_start(oner_sb[:], oner[:])
            nc.sync.dma_start(wqk_sb[:], wqk[:])
            nc.sync.dma_start(ident_sb[:], identd[:])
            nc.sync.dma_start(bias_sb[:], biasd[:])

            # ---- Phase Q: q projection + fused RMS norm --------------------
            with tc.tile_pool(name="qp", bufs=3) as qp, \
                 tc.tile_pool(name="qps", bufs=1, space="PSUM") as qps, \
                 tc.tile_pool(name="qps2", bufs=1, space="PSUM") as qps2:
                for half in range(2):
                    tsl = slice(half * 512, (half + 1) * 512)
                    psq = [qps.tile([P, 512], F32, tag=f"q{f}",
                                    name=f"psq{f}") for f in range(HPC)]
                    for qr in range(8):
                        wq_q = qp.tile([P, 4, 512], F16, tag="wqq", name="wqq")
                        nc.sync.dma_start(wq_q[:],
                                          wqT[:, qr * 4:(qr + 1) * 4, :])
                        ht_q = qp.tile([P, 4, 512], F16, tag="htq", name="htq")
                        nc.sync.dma_start(ht_q[:],
                                          hT[:, half, qr * 4:(qr + 1) * 4, :])
                        for cc in range(4):
                            for f in range(HPC):
                                nc.tensor.matmul(
                                    psq[f][:],
                                    wq_q[:, cc, f * P:(f + 1) * P],
                                    ht_q[:, cc, :],
                                    start=(qr == 0 and cc == 0),
                                    stop=(qr == 7 and cc == 3))
                    for f in range(HPC):
                        qsq = qp.tile([P, 512], F16, tag="qsq", name="qsq")
                        nc.vector.tensor_copy(qT[:, f, tsl], psq[f][:])
                        nc.vector.tensor_tensor(qsq[:], qT[:, f, tsl],
                                                qT[:, f, tsl], MUL)
                        pss = qps2.tile([1, 512], F32, tag="pss", name="pssq")
                        nc.tensor.matmul(pss[:], onec_sb[:], qsq[:],
                                         start=True, stop=True)
                        sq = qp.tile([1, 512], F32, tag="sq", name="sq")
                        nc.scalar.activation(sq[:], pss[:], AF.Sqrt,
                                             bias=eps_sb[0:1], scale=1.0 / HD)
                        nc.vector.reciprocal(sq[:], sq[:])
                        psb = qps2.tile([P, 512], F32, tag="psb", name="psbq")
                        nc.tensor.matmul(psb[:], wqk_sb[:], sq[:],
                                         start=True, stop=True)
                        nc.vector.tensor_tensor(qT[:, f, tsl], qT[:, f, tsl],
                                                psb[:], MUL)

            # ---- pools for KV / attention / o_proj -------------------------
            kvw_cm = tc.tile_pool(name="kvw", bufs=1)
            kvw = kvw_cm.__enter__()
            ctp_cm = tc.tile_pool(name="ctp", bufs=6)
            ctp = ctp_cm.__enter__()
            kvp_cm = tc.tile_pool(name="kvp", bufs=2)
            kvp = kvp_cm.__enter__()
            ep_cm = tc.tile_pool(name="ep", bufs=2)
            ep = ep_cm.__enter__()
            accp_cm = tc.tile_pool(name="accp", bufs=2)
            accp = accp_cm.__enter__()
            op_cm = tc.tile_pool(name="op", bufs=2)
            op = op_cm.__enter__()
            pssp_cm = tc.tile_pool(name="pssp", bufs=2, space="PSUM")
            pssp = pssp_cm.__enter__()
            psap_cm = tc.tile_pool(name="psap", bufs=1, space="PSUM")
            psap = psap_cm.__enter__()
            psro_cm = tc.tile_pool(name="psro", bufs=1, space="PSUM")
            psro = psro_cm.__enter__()
            kvps_cm = tc.tile_pool(name="kvps", bufs=1, space="PSUM")
            kvps = kvps_cm.__enter__()

            wkv_sb = kvw.tile([P, NC, 2 * P], F16)
            nc.sync.dma_start(wkv_sb[:], wkv[:])

            pso_pool = [psro]

            def unit_gen(sg, qc0, hp):
                t0, nt = sg["t0"], sg["nt"]
                qc1 = min(qc0 + 256, sg["q1"])
                nq = qc1 - qc0
                nqp = nq + (nq & 1)
                h0 = 2 * hp
                E = ep.tile([P, nt, 2, nqp], F16, tag="E", name="E")
                for ti in range(nt):
                    t = t0 + ti
                    pss = pssp.tile([P, 2, nqp], F32, tag="pss", name="pss")
                    nc.tensor.matmul(pss[:], kT[:, t, :],
                                     qT[:, h0:h0 + 2, qc0:qc0 + nqp],
                                     start=True, stop=True)
                    if sg["rem"] and ti == nt - 1:
                        nc.scalar.activation(E[:, ti, :, :], pss[:], AF.Exp,
                                             bias=bias_sb[:,
                                                          sg["i"]:sg["i"] + 1],
                                             scale=A_sb[:, t:t + 1])
                    else:
                        nc.scalar.activation(E[:, ti, :, :], pss[:], AF.Exp,
                                             scale=A_sb[:, t:t + 1])
                    if ti % 7 == 6:
                        yield
                yield
                psa = psap.tile([P, 2, nqp], F32, tag="psa", name="psa")
                acc = accp.tile([P, 2, nqp], F16, tag="acc", name="acc")
                for ti in range(nt):
                    nc.tensor.matmul(psa[:], vN[:, t0 + ti, :],
                                     E[:, ti, :, :],
                                     start=(ti == 0), stop=(ti == nt - 1))
                    # DVE accumulation tree for the softmax denominator runs
                    # alongside the attnV matmuls
                    if ti == 0:
                        if nt == 1:
                            nc.vector.tensor_copy(acc[:], E[:, 0, :, :])
                    elif ti == 1:
                        nc.vector.tensor_tensor(acc[:], E[:, 0, :, :],
                                                E[:, 1, :, :], ADD)
                    else:
                        nc.vector.tensor_tensor(acc[:], acc[:],
                                                E[:, ti, :, :], ADD)
                    if ti == nt // 2:
                        yield
                # partition-sum + broadcast of 1/den via two small matmuls,
                # borrowing rotation slots from the pss tag (no extra banks)
                psd = pssp.tile([1, 2 * nqp], F32, tag="pss", name="psd")
                nc.tensor.matmul(psd[:], onec_sb[:], acc[:],
                                 start=True, stop=True)
                rden = accp.tile([1, 2 * nqp], F32, tag="rden", name="rden")
                nc.vector.reciprocal(rden[:], psd[:])
                yield
                psb = pssp.tile([P, 2 * nqp], F32, tag="pss", name="psb")
                nc.tensor.matmul(psb[:], oner_sb[:], rden[:],
                                 start=True, stop=True)
                bden = accp.tile([P, 2 * nqp], F16, tag="bden", name="bden")
                nc.vector.tensor_copy(bden[:], psb[:])
                for j in range(2):
                    nc.vector.tensor_tensor(aT[:, h0 + j, qc0:qc1],
                                            psa[:, j, 0:nq],
                                            bden[:, j * nqp:j * nqp + nq],
                                            MUL)

            def unit_gen_special(sg, qc0, hp):
                # degenerate segment (empty kv): uniform attention over ALL
                # real kpos.  E = 1 on real rows, 0 on pad rows, spanning
                # every segment's tiles.
                qc1 = min(qc0 + 256, sg["q1"])
                nq = qc1 - qc0
                nqp = nq + (nq & 1)
                h0 = 2 * hp
                psa = psap.tile([P, 2, nqp], F32, tag="psa", name="psa")
                acc = accp.tile([P, 2, nqp], F16, tag="acc", name="acc")
                first = True
                tlist = []
                for s2 in segs:
                    for ti in range(s2["nt"]):
                        full = not (s2["rem"] and ti == s2["nt"] - 1)
                        tlist.append((s2["t0"] + ti,
                                      P if full else s2["rem"]))
                CH = 13
                for c0 in range(0, len(tlist), CH):
                    chunk = tlist[c0:c0 + CH]
                    E = ep.tile([P, len(chunk), 2, nqp], F16, tag="E",
                                name="E")
                    for j, (t, hi) in enumerate(chunk):
                        nc.sync.dma_start(E[0:hi, j, :, :],
                                          onesd[0:hi, :2 * nqp])
                        if hi < P:
                            nc.sync.dma_start(E[hi:P, j, :, :],
                                              zerosd[hi:P, :2 * nqp])
                    for j, (t, hi) in enumerate(chunk):
                        last = (c0 + CH >= len(tlist) and j == len(chunk) - 1)
                        nc.tensor.matmul(psa[:], vN[:, t, :], E[:, j, :, :],
                                         start=(c0 == 0 and j == 0),
                                         stop=last)
                    for j in range(len(chunk)):
                        if c0 == 0 and j == 0:
                            nc.vector.tensor_copy(acc[:], E[:, 0, :, :])
                        else:
                            nc.vector.tensor_tensor(acc[:], acc[:],
                                                    E[:, j, :, :], ADD)
                    yield
                psd = pssp.tile([1, 2 * nqp], F32, tag="pss", name="psd")
                nc.tensor.matmul(psd[:], onec_sb[:], acc[:],
                                 start=True, stop=True)
                rden = accp.tile([1, 2 * nqp], F32, tag="rden", name="rden")
                nc.vector.reciprocal(rden[:], psd[:])
                psb = pssp.tile([P, 2 * nqp], F32, tag="pss", name="psb")
                nc.tensor.matmul(psb[:], oner_sb[:], rden[:],
                                 start=True, stop=True)
                bden = accp.tile([P, 2 * nqp], F16, tag="bden", name="bden")
                nc.vector.tensor_copy(bden[:], psb[:])
                for j in range(2):
                    nc.vector.tensor_tensor(aT[:, h0 + j, qc0:qc1],
                                            psa[:, j, 0:nq],
                                            bden[:, j * nqp:j * nqp + nq],
                                            MUL)

            def oproj_gen(qt):
                osb = op.tile([P, 8, 512], F16, tag="osb", name="osb")
                for n8 in range(8):
                    pso = pso_pool[0].tile([P, 512], F32, tag="pso",
                                           name="pso")
                    for co in range(HPC):
                        nc.tensor.matmul(pso[:],
                                         aT[:, co, qt * P:(qt + 1) * P],
                                         wo_sb[:, n8, co, :],
                                         start=(co == 0),
                                         stop=(co == HPC - 1))
                    nc.vector.tensor_copy(osb[:, n8, :], pso[:])
                    if n8 < 7:
                        yield
                nc.sync.dma_start(out[qt * P:(qt + 1) * P, :], osb[:])

            # bookkeeping
            NQT = TQ // P
            qt_segs = [set() for _ in range(NQT)]
            for sg in segs:
                for qt in range(sg["q0"] // P, (sg["q1"] + P - 1) // P):
                    qt_segs[qt].add(sg["i"])
            unit_total = {sg["i"]: 0 for sg in segs}
            unit_done = {sg["i"]: 0 for sg in segs}
            for sg in segs:
                for _ in range(sg["q0"], sg["q1"], 256):
                    unit_total[sg["i"]] += HPC // 2
            o_enq = [False] * NQT
            aq = []  # [(seg_i, generator)]
            oq = []  # [generator]
            opending = []  # oproj gens held until the next slab boundary

            def finish_unit(si):
                unit_done[si] += 1
                for qt in range(NQT):
                    if o_enq[qt]:
                        continue
                    if all(unit_done[s2] == unit_total[s2]
                           for s2 in qt_segs[qt]):
                        o_enq[qt] = True
                        opending.append(oproj_gen(qt))

            def advance_attn():
                si, g = aq[0]
                try:
                    next(g)
                except StopIteration:
                    aq.pop(0)
                    finish_unit(si)

            def advance_oproj():
                g = oq[0]
                try:
                    next(g)
                except StopIteration:
                    oq.pop(0)

            def drip():
                if aq:
                    advance_attn()
                elif oq:
                    advance_oproj()

            def enqueue_ready(kmax_t):
                for sg in segs:
                    if sg.get("enq") or sg["special"]:
                        continue
                    if sg["t0"] + sg["nt"] <= kmax_t:
                        sg["enq"] = True
                        for qc0 in range(sg["q0"], sg["q1"], 256):
                            for hp in range(HPC // 2):
                                aq.append((sg["i"], unit_gen(sg, qc0, hp)))

            # ---- KV slabs with drip-fed attention/o_proj -------------------
            for s in range(SLABS):
                psk = kvps.tile([P, 512], F32, tag="psk", name="psk")
                psv = kvps.tile([P, 512], F32, tag="psv", name="psv")
                for q4 in range(8):
                    ct_q = ctp.tile([P, 4, 512], F16, tag="ctq", name="ctq")
                    nc.sync.dma_start(ct_q[:],
                                      cT[:, s, q4 * 4:(q4 + 1) * 4, :])
                    for cc in range(4):
                        c = q4 * 4 + cc
                        nc.tensor.matmul(psk[:], wkv_sb[:, c, 0:P],
                                         ct_q[:, cc, :],
                                         start=(c == 0), stop=(c == NC - 1))
                        nc.tensor.matmul(psv[:], wkv_sb[:, c, P:2 * P],
                                         ct_q[:, cc, :],
                                         start=(c == 0), stop=(c == NC - 1))
                    drip()
                    if len(aq) + len(oq) > 6:
                        drip()
                # slab drain: k copy + rms stats + v transposes
                nc.vector.tensor_copy(kT[:, 4 * s:4 * s + 4, :], psk[:])
                ksq = kvp.tile([P, 512], F16, tag="ksq", name="ksq")
                kslab = kT[:, 4 * s:4 * s + 4, :]
                nc.vector.tensor_tensor(ksq[:], kslab, kslab, MUL)
                vstage = kvp.tile([P, 512], F16, tag="vstage", name="vstage")
                nc.vector.tensor_copy(vstage[:], psv[:])
                sqk4 = kvp.tile([P, 4], F32, tag="sqk4", name="sqk4")
                for t in range(4):
                    psr = psro.tile([P, 2], F32, tag="psr", name="psr")
                    nc.tensor.matmul(psr[:], ksq[:, t * P:(t + 1) * P],
                                     onec2_sb[:], start=True, stop=True)
                    nc.vector.tensor_copy(sqk4[:, t:t + 1], psr[:, 0:1])
                    pst = kvps.tile([P, P], F16, tag="pst", name="pst")
                    nc.tensor.transpose(pst[:], vstage[:, t * P:(t + 1) * P],
                                        ident_sb[:])
                    nc.vector.tensor_copy(vN[:, 4 * s + t, :], pst[:])
                    if t % 2 == 1:
                        drip()
                sqk4b = kvp.tile([P, 4], F32, tag="sqk4b", name="sqk4b")
                nc.scalar.activation(sqk4b[:], sqk4[:], AF.Sqrt,
                                     bias=eps_sb[:], scale=1.0 / HD)
                nc.vector.reciprocal(A_sb[:, 4 * s:4 * s + 4], sqk4b[:])
                enqueue_ready(4 * (s + 1))
                oq.extend(opending)
                del opending[:]
                if s == 0:
                    nc.sync.dma_start(wo_sb[:], woT[:])

            # ---- tail: close KV psum, widen o_proj psum --------------------
            kvps_cm.__exit__(None, None, None)
            psro_cm.__exit__(None, None, None)
            pso2_cm = tc.tile_pool(name="pso2", bufs=2, space="PSUM")
            pso2 = pso2_cm.__enter__()
            pso_pool[0] = pso2

            if any_special:
                for sg in segs:
                    if sg["special"] and not sg.get("enq"):
                        sg["enq"] = True
                        for qc0 in range(sg["q0"], sg["q1"], 256):
                            for hp in range(HPC // 2):
                                aq.append((sg["i"],
                                           unit_gen_special(sg, qc0, hp)))

            while aq:
                si, g = aq.pop(0)
                try:
                    next(g)
                    aq.append((si, g))
                except StopIteration:
                    finish_unit(si)
                oq.extend(opending)
                del opending[:]
                if oq:
                    advance_oproj()
            oq.extend(opending)
            del opending[:]
            while oq:
                advance_oproj()

            assert all(o_enq), "o_proj bookkeeping failed"

            pso2_cm.__exit__(None, None, None)
            psap_cm.__exit__(None, None, None)
            pssp_cm.__exit__(None, None, None)
            op_cm.__exit__(None, None, None)
            accp_cm.__exit__(None, None, None)
            ep_cm.__exit__(None, None, None)
            kvp_cm.__exit__(None, None, None)
            ctp_cm.__exit__(None, None, None)
            kvw_cm.__exit__(None, None, None)

    nc.finalize()
    return nc, SLABS


def _prepare(inputs):
    gi = {k: np.asarray(v) for k, v in inputs.items()}
    hs = np.ascontiguousarray(gi["hidden_states"], dtype=np.float16)
    cs = np.ascontiguousarray(gi["cross_attention_states"], dtype=np.float16)
    Wq = np.ascontiguousarray(gi["Wq"], dtype=np.float16)
    Wk = np.ascontiguousarray(gi["Wk"], dtype=np.float16)
    Wv = np.ascontiguousarray(gi["Wv"], dtype=np.float16)
    Wo = np.ascontiguousarray(gi["Wo"], dtype=np.float16)
    qw = np.asarray(gi["q_norm_w"], dtype=np.float32).reshape(-1)
    kw = np.asarray(gi["k_norm_w"], dtype=np.float32).reshape(-1)
    cu_q = np.asarray(gi["cu_seqlen_q"]).reshape(-1)
    cu_k = np.asarray(gi["cu_seqlen_k"]).reshape(-1)

    segs, TT = _segments(cu_q, cu_k)
    nc, SLABS = _build(segs, TT)
    TKP = SLABS * 512

    # packed layouts: partition-major with long contiguous per-partition runs
    hTd = np.ascontiguousarray(
        hs.T.reshape(NC, P, 2, 512).transpose(1, 2, 0, 3))   # [128,2,32,512]
    # per-segment padded kpos stream
    cTp = np.zeros((HIDDEN, TKP), np.float16)
    for sg in segs:
        o = sg["t0"] * P
        cTp[:, o:o + sg["k1"] - sg["k0"]] = cs[sg["k0"]:sg["k1"]].T
    cTd = np.ascontiguousarray(
        cTp.reshape(NC, P, SLABS, 512).transpose(1, 2, 0, 3))
    wqkv = (qw * kw * SCALE).reshape(1, P).astype(np.float32)
    onec = np.ones((P, 1), np.float16)
    onec2 = np.ones((P, 2), np.float16)
    onerv = np.ones((1, P), np.float32)
    ident = np.eye(P, dtype=np.float16)
    biasv = np.zeros((P, len(segs)), np.float32)
    for sg in segs:
        if sg["rem"]:
            biasv[sg["rem"]:, sg["i"]] = -50.0
    zeros = np.zeros((P, 512), np.float16)
    ones = np.ones((P, 512), np.float16)

    in_maps = []
    for c in range(NCORES):
        fsl = slice(c * P * HPC, (c + 1) * P * HPC)
        ksl = slice(c * P, (c + 1) * P)
        wq_d = np.ascontiguousarray(
            Wq[fsl, :].T.reshape(NC, P, P * HPC).transpose(1, 0, 2))
        wkv_d = np.concatenate([
            Wk[ksl, :].T.reshape(NC, P, P).transpose(1, 0, 2),
            Wv[ksl, :].T.reshape(NC, P, P).transpose(1, 0, 2)], axis=2)
        wo_d = np.ascontiguousarray(
            Wo[:, fsl].T.reshape(HPC, P, 8, 512).transpose(1, 2, 0, 3))
        in_maps.append({
            "hT": hTd,
            "cT": cTd,
            "wqT": wq_d,
            "wkv": np.ascontiguousarray(wkv_d),
            "woT": wo_d,
            "wqk": wqkv,
            "onec": onec,
            "onec2": onec2,
            "oner": onerv,
            "identd": ident,
            "biasd": biasv,
            "zerosd": zeros,
            "onesd": ones,
        })

    return nc, in_maps


def _reduce(results) -> np.ndarray:
    o = np.zeros((TQ, HIDDEN), np.float64)
    for c in range(NCORES):
        o += results[c]["o"].astype(np.float64)
    return o.astype(np.float32)


def kernel(**inputs) -> np.ndarray:
    from concourse.bass_utils import run_bass_kernel_spmd

    nc, in_maps = _prepare(inputs)
    r = run_bass_kernel_spmd(nc, in_maps, list(range(NCORES)))
    return _reduce(r.results)


# revision 24
# speedup vs baseline: 1.1285x; 1.0145x over previous
"""MllamaTextCrossAttention kernel for 8 Trainium2 NeuronCores.

Strategy: tensor-parallel over heads (4 q-heads + 1 kv-head per core).
Each core computes q/k/v projections for its shard, fused QK-RMS-norm,
block-diagonal varlen attention (segments baked in at build time from the
actual cu_seqlen values), and a row-parallel o_proj partial of the full
[1024, 4096] output. The host sums the 8 partials.

Layout: cross_attention_states is repacked with PER-SEGMENT padding so every
segment spans a whole number of 128-kpos tiles; pad positions are masked in
the softmax via a per-partition bias of -50 on the exp activation (exp -> 0),
so no boundary patch DMAs are needed.  All attention runs in transposed
layout (features on partitions): qT [d, tok], kT [d, kpos], scoresT
[kpos, q], attnT [d, q].  Softmax denominators use a DVE accumulation tree +
GpSimd partition_all_reduce (no PE work, no PSUM).  Attention units and
o_proj chains are drip-fed between KV-projection matmul groups so the PE
queue never drains (keeps HAM at 8/8).
"""
import os
import sys

if "/opt/trn_rl_repo" not in sys.path:
    sys.path.insert(0, "/opt/trn_rl_repo")

import numpy as np

HIDDEN = 4096
N_HEADS = 32
N_KV = 8
HD = 128
EPS = 1e-5
SCALE = HD ** -0.5
TQ = 1024
TK = 6404
NCORES = 8
HPC = N_HEADS // NCORES  # 4 q-heads per core
P = 128
NC = HIDDEN // P         # 32 contraction chunks


def _segments(cu_q, cu_k):
    eq = [0] + [int(min(max(int(v), 0), TQ)) for v in cu_q] + [TQ]
    ek = [0] + [int(min(max(int(v), 0), TK)) for v in cu_k] + [TK]
    segs = []
    for i in range(len(eq) - 1):
        q0, q1 = eq[i], eq[i + 1]
        k0, k1 = ek[i], ek[i + 1]
        if q1 <= q0:
            continue
        if k1 <= k0:
            segs.append(dict(q0=q0, q1=q1, k0=0, k1=TK, special=True))
        else:
            segs.append(dict(q0=q0, q1=q1, k0=k0, k1=k1, special=False))
    t = 0
    for i, sg in enumerate(segs):
        sg["i"] = i
        sg["t0"] = t
        sg["nt"] = (sg["k1"] - sg["k0"] + P - 1) // P
        sg["rem"] = (sg["k1"] - sg["k0"]) % P  # real rows in last tile (0=full)
        t += sg["nt"]
    return segs, t


def _build(segs, TT):
    import concourse.bass as bass  # noqa: F401
    import concourse.tile as tile
    from concourse import bacc, mybir
    from concourse.bass_isa import ReduceOp

    F32 = mybir.dt.float32
    F16 = mybir.dt.float16
    AF = mybir.ActivationFunctionType
    MUL = mybir.AluOpType.mult
    ADD = mybir.AluOpType.add

    SLABS = (TT + 3) // 4          # 512-kpos slabs over the padded stream
    NTI = SLABS * 4                # 128-kpos tiles (incl. tail pad tiles)
    NSEG = len(segs)

    nc = bacc.Bacc("TRN2", target_bir_lowering=False, debug=False,
                   num_devices=NCORES)

    hT = nc.declare_dram_parameter("hT", [P, 2, NC, 512], F16, isOutput=False)
    cT = nc.declare_dram_parameter("cT", [P, SLABS, NC, 512], F16,
                                   isOutput=False)
    wqT = nc.declare_dram_parameter("wqT", [P, NC, P * HPC], F16,
                                    isOutput=False)
    wkv = nc.declare_dram_parameter("wkv", [P, NC, 2 * P], F16, isOutput=False)
    woT = nc.declare_dram_parameter("woT", [P, 8, HPC, 512], F16,
                                    isOutput=False)
    wqk = nc.declare_dram_parameter("wqk", [1, P], F32, isOutput=False)
    onec = nc.declare_dram_parameter("onec", [P, 1], F16, isOutput=False)
    onec2 = nc.declare_dram_parameter("onec2", [P, 2], F16, isOutput=False)
    oner = nc.declare_dram_parameter("oner", [1, P], F32, isOutput=False)
    identd = nc.declare_dram_parameter("identd", [P, P], F16, isOutput=False)
    biasd = nc.declare_dram_parameter("biasd", [P, NSEG], F32, isOutput=False)
    zerosd = nc.declare_dram_parameter("zerosd", [P, 512], F16, isOutput=False)
    onesd = nc.declare_dram_parameter("onesd", [P, 512], F16, isOutput=False)
    out = nc.declare_dram_parameter("o", [TQ, HIDDEN], F16, isOutput=True)

    any_special = any(sg["special"] for sg in segs)

    with tile.TileContext(nc) as tc:
        with tc.tile_pool(name="persist", bufs=1) as pp:
            qT = pp.tile([P, HPC, TQ + 2], F16)     # q transposed (+2 pad)
            kT = pp.tile([P, NTI, P], F16)          # k transposed, 128-blocks
            vN = pp.tile([P, NTI, P], F16)          # v natural, 128-blocks
            aT = pp.tile([P, HPC, TQ], F16)         # attn output transposed
            A_sb = pp.tile([P, NTI], F32)           # 1/rms_k per kpos tile
            wo_sb = pp.tile([P, 8, HPC, 512], F16)  # o_proj weights, resident
            onec_sb = pp.tile([P, 1], F16)
            onec2_sb = pp.tile([P, 2], F16)
            oner_sb = pp.tile([1, P], F32)
            wqk_sb = pp.tile([1, P], F32)
            ident_sb = pp.tile([P, P], F16)
            bias_sb = pp.tile([P, NSEG], F32)
            eps_sb = pp.tile([P, 1], F32)
            nc.vector.memset(eps_sb[:], EPS)
            nc.sync.dma_start(onec_sb[:], onec[:])
            nc.sync.dma_start(onec2_sb[:], onec2[:])
            nc.sync.dma_start(oner_sb[:], oner[:])
            nc.sync.dma_start(wqk_sb[:], wqk[:])
            nc.sync.dma_start(ident_sb[:], identd[:])
            nc.sync.dma_start(bias_sb[:], biasd[:])

            # pools that outlive the Q phase but whose DMAs must interleave
            # with the Q-phase stream (wkv + first cT chunks prefetch)
            kvw_cm = tc.tile_pool(name="kvw", bufs=1)
            kvw = kvw_cm.__enter__()
            ctp_cm = tc.tile_pool(name="ctp", bufs=6)
            ctp = ctp_cm.__enter__()
            wkv_sb = kvw.tile([P, NC, 2 * P], F16)
            prefetched = {}

            # ---- Phase Q: q projection + fused RMS norm --------------------
            with tc.tile_pool(name="qp", bufs=3) as qp, \
                 tc.tile_pool(name="qps", bufs=1, space="PSUM") as qps, \
                 tc.tile_pool(name="qps2", bufs=1, space="PSUM") as qps2:
                for half in range(2):
                    tsl = slice(half * 512, (half + 1) * 512)
                    psq = [qps.tile([P, 512], F32, tag=f"q{f}",
                                    name=f"psq{f}") for f in range(HPC)]
                    for qr in range(8):
                        wq_q = qp.tile([P, 4, 512], F16, tag="wqq", name="wqq")
                        nc.sync.dma_start(wq_q[:],
                                          wqT[:, qr * 4:(qr + 1) * 4, :])
                        ht_q = qp.tile([P, 4, 512], F16, tag="htq", name="htq")
                        nc.sync.dma_start(ht_q[:],
                                          hT[:, half, qr * 4:(qr + 1) * 4, :])
                        if half == 0 and qr == 1:
                            nc.sync.dma_start(wkv_sb[:], wkv[:])
                        if half == 0 and 2 <= qr <= 7:
                            pi = qr - 2
                            pct = ctp.tile([P, 4, 512], F16, tag="ctq",
                                           name="ctq")
                            nc.sync.dma_start(
                                pct[:], cT[:, 0, pi * 4:(pi + 1) * 4, :])
                            prefetched[(0, pi)] = pct
                        for cc in range(4):
                            for f in range(HPC):
                                nc.tensor.matmul(
                                    psq[f][:],
                                    wq_q[:, cc, f * P:(f + 1) * P],
                                    ht_q[:, cc, :],
                                    start=(qr == 0 and cc == 0),
                                    stop=(qr == 7 and cc == 3))
                    for f in range(HPC):
                        qsq = qp.tile([P, 512], F16, tag="qsq", name="qsq")
                        nc.vector.tensor_copy(qT[:, f, tsl], psq[f][:])
                        nc.vector.tensor_tensor(qsq[:], qT[:, f, tsl],
                                                qT[:, f, tsl], MUL)
                        pss = qps2.tile([1, 512], F32, tag="pss", name="pssq")
                        nc.tensor.matmul(pss[:], onec_sb[:], qsq[:],
                                         start=True, stop=True)
                        sq = qp.tile([1, 512], F32, tag="sq", name="sq")
                        nc.scalar.activation(sq[:], pss[:], AF.Sqrt,
                                             bias=eps_sb[0:1], scale=1.0 / HD)
                        nc.vector.reciprocal(sq[:], sq[:])
                        psb = qps2.tile([P, 512], F32, tag="psb", name="psbq")
                        nc.tensor.matmul(psb[:], wqk_sb[:], sq[:],
                                         start=True, stop=True)
                        nc.vector.tensor_tensor(qT[:, f, tsl], qT[:, f, tsl],
                                                psb[:], MUL)

            # ---- pools for KV / attention / o_proj -------------------------
            kvp_cm = tc.tile_pool(name="kvp", bufs=2)
            kvp = kvp_cm.__enter__()
            ep_cm = tc.tile_pool(name="ep", bufs=2)
            ep = ep_cm.__enter__()
            accp_cm = tc.tile_pool(name="accp", bufs=2)
            accp = accp_cm.__enter__()
            op_cm = tc.tile_pool(name="op", bufs=2)
            op = op_cm.__enter__()
            pssp_cm = tc.tile_pool(name="pssp", bufs=2, space="PSUM")
            pssp = pssp_cm.__enter__()
            psap_cm = tc.tile_pool(name="psap", bufs=1, space="PSUM")
            psap = psap_cm.__enter__()
            psro_cm = tc.tile_pool(name="psro", bufs=1, space="PSUM")
            psro = psro_cm.__enter__()
            kvps_cm = tc.tile_pool(name="kvps", bufs=1, space="PSUM")
            kvps = kvps_cm.__enter__()

            pso_pool = [psro]

            def unit_gen(sg, qc0, hp):
                t0, nt = sg["t0"], sg["nt"]
                qc1 = min(qc0 + 256, sg["q1"])
                nq = qc1 - qc0
                nqp = nq + (nq & 1)
                h0 = 2 * hp
                E = ep.tile([P, nt, 2, nqp], F16, tag="E", name="E")
                for ti in range(nt):
                    t = t0 + ti
                    pss = pssp.tile([P, 2, nqp], F32, tag="pss", name="pss")
                    nc.tensor.matmul(pss[:], kT[:, t, :],
                                     qT[:, h0:h0 + 2, qc0:qc0 + nqp],
                                     start=True, stop=True)
                    if sg["rem"] and ti == nt - 1:
                        nc.scalar.activation(E[:, ti, :, :], pss[:], AF.Exp,
                                             bias=bias_sb[:,
                                                          sg["i"]:sg["i"] + 1],
                                             scale=A_sb[:, t:t + 1])
                    else:
                        nc.scalar.activation(E[:, ti, :, :], pss[:], AF.Exp,
                                             scale=A_sb[:, t:t + 1])
                    if ti % 7 == 6:
                        yield
                yield
                psa = psap.tile([P, 2, nqp], F32, tag="psa", name="psa")
                acc = accp.tile([P, 2, nqp], F16, tag="acc", name="acc")
                for ti in range(nt):
                    nc.tensor.matmul(psa[:], vN[:, t0 + ti, :],
                                     E[:, ti, :, :],
                                     start=(ti == 0), stop=(ti == nt - 1))
                    # DVE accumulation tree for the softmax denominator runs
                    # alongside the attnV matmuls
                    if ti == 0:
                        if nt == 1:
                            nc.vector.tensor_copy(acc[:], E[:, 0, :, :])
                    elif ti == 1:
                        nc.vector.tensor_tensor(acc[:], E[:, 0, :, :],
                                                E[:, 1, :, :], ADD)
                    else:
                        nc.vector.tensor_tensor(acc[:], acc[:],
                                                E[:, ti, :, :], ADD)
                    if ti == nt // 2:
                        yield
                # partition-sum + broadcast of 1/den via two small matmuls,
                # borrowing rotation slots from the pss tag (no extra banks)
                psd = pssp.tile([1, 2 * nqp], F32, tag="pss", name="psd")
                nc.tensor.matmul(psd[:], onec_sb[:], acc[:],
                                 start=True, stop=True)
                rden = accp.tile([1, 2 * nqp], F32, tag="rden", name="rden")
                nc.vector.reciprocal(rden[:], psd[:])
                yield
                psb = pssp.tile([P, 2 * nqp], F32, tag="pss", name="psb")
                nc.tensor.matmul(psb[:], oner_sb[:], rden[:],
                                 start=True, stop=True)
                bden = accp.tile([P, 2 * nqp], F16, tag="bden", name="bden")
                nc.vector.tensor_copy(bden[:], psb[:])
                for j in range(2):
                    nc.vector.tensor_tensor(aT[:, h0 + j, qc0:qc1],
                                            psa[:, j, 0:nq],
                                            bden[:, j * nqp:j * nqp + nq],
                                            MUL)

            def unit_gen_special(sg, qc0, hp):
                # degenerate segment (empty kv): uniform attention over ALL
                # real kpos.  E = 1 on real rows, 0 on pad rows, spanning
                # every segment's tiles.
                qc1 = min(qc0 + 256, sg["q1"])
                nq = qc1 - qc0
                nqp = nq + (nq & 1)
                h0 = 2 * hp
                psa = psap.tile([P, 2, nqp], F32, tag="psa", name="psa")
                acc = accp.tile([P, 2, nqp], F16, tag="acc", name="acc")
                first = True
                tlist = []
                for s2 in segs:
                    for ti in range(s2["nt"]):
                        full = not (s2["rem"] and ti == s2["nt"] - 1)
                        tlist.append((s2["t0"] + ti,
                                      P if full else s2["rem"]))
                CH = 13
                for c0 in range(0, len(tlist), CH):
                    chunk = tlist[c0:c0 + CH]
                    E = ep.tile([P, len(chunk), 2, nqp], F16, tag="E",
                                name="E")
                    for j, (t, hi) in enumerate(chunk):
                        nc.sync.dma_start(E[0:hi, j, :, :],
                                          onesd[0:hi, :2 * nqp])
                        if hi < P:
                            nc.sync.dma_start(E[hi:P, j, :, :],
                                              zerosd[hi:P, :2 * nqp])
                    for j, (t, hi) in enumerate(chunk):
                        last = (c0 + CH >= len(tlist) and j == len(chunk) - 1)
                        nc.tensor.matmul(psa[:], vN[:, t, :], E[:, j, :, :],
                                         start=(c0 == 0 and j == 0),
                                         stop=last)
                    for j in range(len(chunk)):
                        if c0 == 0 and j == 0:
                            nc.vector.tensor_copy(acc[:], E[:, 0, :, :])
                        else:
                            nc.vector.tensor_tensor(acc[:], acc[:],
                                                    E[:, j, :, :], ADD)
                    yield
                psd = pssp.tile([1, 2 * nqp], F32, tag="pss", name="psd")
                nc.tensor.matmul(psd[:], onec_sb[:], acc[:],
                                 start=True, stop=True)
                rden = accp.tile([1, 2 * nqp], F32, tag="rden", name="rden")
                nc.vector.reciprocal(rden[:], psd[:])
                psb = pssp.tile([P, 2 * nqp], F32, tag="pss", name="psb")
                nc.tensor.matmul(psb[:], oner_sb[:], rden[:],
                                 start=True, stop=True)
                bden = accp.tile([P, 2 * nqp], F16, tag="bden", name="bden")
                nc.vector.tensor_copy(bden[:], psb[:])
                for j in range(2):
                    nc.vector.tensor_tensor(aT[:, h0 + j, qc0:qc1],
                                            psa[:, j, 0:nq],
                                            bden[:, j * nqp:j * nqp + nq],
                                            MUL)

            def oproj_gen(qt):
                osb = op.tile([P, 8, 512], F16, tag="osb", name="osb")
                for n8 in range(8):
                    pso = pso_pool[0].tile([P, 512], F32, tag="pso",
                                           name="pso")
                    for co in range(HPC):
                        nc.tensor.matmul(pso[:],
                                         aT[:, co, qt * P:(qt + 1) * P],
                                         wo_sb[:, n8, co, :],
                                         start=(co == 0),
                                         stop=(co == HPC - 1))
                    nc.vector.tensor_copy(osb[:, n8, :], pso[:])
                    if n8 < 7:
                        yield
                nc.sync.dma_start(out[qt * P:(qt + 1) * P, :], osb[:])

            # bookkeeping
            NQT = TQ // P
            qt_segs = [set() for _ in range(NQT)]
            for sg in segs:
                for qt in range(sg["q0"] // P, (sg["q1"] + P - 1) // P):
                    qt_segs[qt].add(sg["i"])
            unit_total = {sg["i"]: 0 for sg in segs}
            unit_done = {sg["i"]: 0 for sg in segs}
            for sg in segs:
                for _ in range(sg["q0"], sg["q1"], 256):
                    unit_total[sg["i"]] += HPC // 2
            o_enq = [False] * NQT
            aq = []  # [(seg_i, generator)]
            oq = []  # [generator]
            opending = []  # oproj gens held until the next slab boundary

            def finish_unit(si):
                unit_done[si] += 1
                for qt in range(NQT):
                    if o_enq[qt]:
                        continue
                    if all(unit_done[s2] == unit_total[s2]
                           for s2 in qt_segs[qt]):
                        o_enq[qt] = True
                        opending.append(oproj_gen(qt))

            def advance_attn():
                si, g = aq[0]
                try:
                    next(g)
                except StopIteration:
                    aq.pop(0)
                    finish_unit(si)

            def advance_oproj():
                g = oq[0]
                try:
                    next(g)
                except StopIteration:
                    oq.pop(0)

            def drip():
                if aq:
                    advance_attn()
                elif oq:
                    advance_oproj()

            def enqueue_ready(kmax_t):
                for sg in segs:
                    if sg.get("enq") or sg["special"]:
                        continue
                    if sg["t0"] + sg["nt"] <= kmax_t:
                        sg["enq"] = True
                        for qc0 in range(sg["q0"], sg["q1"], 256):
                            for hp in range(HPC // 2):
                                aq.append((sg["i"], unit_gen(sg, qc0, hp)))

            # ---- KV slabs with drip-fed attention/o_proj -------------------
            for s in range(SLABS):
                psk = kvps.tile([P, 512], F32, tag="psk", name="psk")
                psv = kvps.tile([P, 512], F32, tag="psv", name="psv")
                for q4 in range(8):
                    ct_q = prefetched.pop((s, q4), None)
                    if ct_q is None:
                        ct_q = ctp.tile([P, 4, 512], F16, tag="ctq",
                                        name="ctq")
                        nc.sync.dma_start(ct_q[:],
                                          cT[:, s, q4 * 4:(q4 + 1) * 4, :])
                    for cc in range(4):
                        c = q4 * 4 + cc
                        nc.tensor.matmul(psk[:], wkv_sb[:, c, 0:P],
                                         ct_q[:, cc, :],
                                         start=(c == 0), stop=(c == NC - 1))
                        nc.tensor.matmul(psv[:], wkv_sb[:, c, P:2 * P],
                                         ct_q[:, cc, :],
                                         start=(c == 0), stop=(c == NC - 1))
                    drip()
                    if len(aq) + len(oq) > 6:
                        drip()
                # slab drain: k copy + rms stats + v transposes
                nc.vector.tensor_copy(kT[:, 4 * s:4 * s + 4, :], psk[:])
                ksq = kvp.tile([P, 512], F16, tag="ksq", name="ksq")
                kslab = kT[:, 4 * s:4 * s + 4, :]
                nc.vector.tensor_tensor(ksq[:], kslab, kslab, MUL)
                vstage = kvp.tile([P, 512], F16, tag="vstage", name="vstage")
                nc.vector.tensor_copy(vstage[:], psv[:])
                sqk4 = kvp.tile([P, 4], F32, tag="sqk4", name="sqk4")
                for t in range(4):
                    psr = psro.tile([P, 2], F32, tag="psr", name="psr")
                    nc.tensor.matmul(psr[:], ksq[:, t * P:(t + 1) * P],
                                     onec2_sb[:], start=True, stop=True)
                    nc.vector.tensor_copy(sqk4[:, t:t + 1], psr[:, 0:1])
                    pst = kvps.tile([P, P], F16, tag="pst", name="pst")
                    nc.tensor.transpose(pst[:], vstage[:, t * P:(t + 1) * P],
                                        ident_sb[:])
                    nc.vector.tensor_copy(vN[:, 4 * s + t, :], pst[:])
                    if t % 2 == 1:
                        drip()
                sqk4b = kvp.tile([P, 4], F32, tag="sqk4b", name="sqk4b")
                nc.scalar.activation(sqk4b[:], sqk4[:], AF.Sqrt,
                                     bias=eps_sb[:], scale=1.0 / HD)
                nc.vector.reciprocal(A_sb[:, 4 * s:4 * s + 4], sqk4b[:])
                enqueue_ready(4 * (s + 1))
                oq.extend(opending)
                del opending[:]
                if s == 0:
                    nc.sync.dma_start(wo_sb[:], woT[:])

            # ---- tail: close KV psum, widen o_proj psum --------------------
            kvps_cm.__exit__(None, None, None)
            psro_cm.__exit__(None, None, None)
            pso2_cm = tc.tile_pool(name="pso2", bufs=2, space="PSUM")
            pso2 = pso2_cm.__enter__()
            pso_pool[0] = pso2

            if any_special:
                for sg in segs:
                    if sg["special"] and not sg.get("enq"):
                        sg["enq"] = True
                        for qc0 in range(sg["q0"], sg["q1"], 256):
                            for hp in range(HPC // 2):
                                aq.append((sg["i"],
                                           unit_gen_special(sg, qc0, hp)))

            while aq:
                si, g = aq.pop(0)
                try:
                    next(g)
                    aq.append((si, g))
                except StopIteration:
                    finish_unit(si)
                oq.extend(opending)
                del opending[:]
                if oq:
                    advance_oproj()
            oq.extend(opending)
            del opending[:]
            while oq:
                advance_oproj()

            assert all(o_enq), "o_proj bookkeeping failed"

            pso2_cm.__exit__(None, None, None)
            psap_cm.__exit__(None, None, None)
            pssp_cm.__exit__(None, None, None)
            op_cm.__exit__(None, None, None)
            accp_cm.__exit__(None, None, None)
            ep_cm.__exit__(None, None, None)
            kvp_cm.__exit__(None, None, None)
            ctp_cm.__exit__(None, None, None)
            kvw_cm.__exit__(None, None, None)

    nc.finalize()
    return nc, SLABS


def _prepare(inputs):
    gi = {k: np.asarray(v) for k, v in inputs.items()}
    hs = np.ascontiguousarray(gi["hidden_states"], dtype=np.float16)
    cs = np.ascontiguousarray(gi["cross_attention_states"], dtype=np.float16)
    Wq = np.ascontiguousarray(gi["Wq"], dtype=np.float16)
    Wk = np.ascontiguousarray(gi["Wk"], dtype=np.float16)
    Wv = np.ascontiguousarray(gi["Wv"], dtype=np.float16)
    Wo = np.ascontiguousarray(gi["Wo"], dtype=np.float16)
    qw = np.asarray(gi["q_norm_w"], dtype=np.float32).reshape(-1)
    kw = np.asarray(gi["k_norm_w"], dtype=np.float32).reshape(-1)
    cu_q = np.asarray(gi["cu_seqlen_q"]).reshape(-1)
    cu_k = np.asarray(gi["cu_seqlen_k"]).reshape(-1)

    segs, TT = _segments(cu_q, cu_k)
    nc, SLABS = _build(segs, TT)
    TKP = SLABS * 512

    # packed layouts: partition-major with long contiguous per-partition runs
    hTd = np.ascontiguousarray(
        hs.T.reshape(NC, P, 2, 512).transpose(1, 2, 0, 3))   # [128,2,32,512]
    # per-segment padded kpos stream
    cTp = np.zeros((HIDDEN, TKP), np.float16)
    for sg in segs:
        o = sg["t0"] * P
        cTp[:, o:o + sg["k1"] - sg["k0"]] = cs[sg["k0"]:sg["k1"]].T
    cTd = np.ascontiguousarray(
        cTp.reshape(NC, P, SLABS, 512).transpose(1, 2, 0, 3))
    wqkv = (qw * kw * SCALE).reshape(1, P).astype(np.float32)
    onec = np.ones((P, 1), np.float16)
    onec2 = np.ones((P, 2), np.float16)
    onerv = np.ones((1, P), np.float32)
    ident = np.eye(P, dtype=np.float16)
    biasv = np.zeros((P, len(segs)), np.float32)
    for sg in segs:
        if sg["rem"]:
            biasv[sg["rem"]:, sg["i"]] = -50.0
    zeros = np.zeros((P, 512), np.float16)
    ones = np.ones((P, 512), np.float16)

    in_maps = []
    for c in range(NCORES):
        fsl = slice(c * P * HPC, (c + 1) * P * HPC)
        ksl = slice(c * P, (c + 1) * P)
        wq_d = np.ascontiguousarray(
            Wq[fsl, :].T.reshape(NC, P, P * HPC).transpose(1, 0, 2))
        wkv_d = np.concatenate([
            Wk[ksl, :].T.reshape(NC, P, P).transpose(1, 0, 2),
            Wv[ksl, :].T.reshape(NC, P, P).transpose(1, 0, 2)], axis=2)
        wo_d = np.ascontiguousarray(
            Wo[:, fsl].T.reshape(HPC, P, 8, 512).transpose(1, 2, 0, 3))
        in_maps.append({
            "hT": hTd,
            "cT": cTd,
            "wqT": wq_d,
            "wkv": np.ascontiguousarray(wkv_d),
            "woT": wo_d,
            "wqk": wqkv,
            "onec": onec,
            "onec2": onec2,
            "oner": onerv,
            "identd": ident,
            "biasd": biasv,
            "zerosd": zeros,
            "onesd": ones,
        })

    return nc, in_maps


def _reduce(results) -> np.ndarray:
    o = np.zeros((TQ, HIDDEN), np.float64)
    for c in range(NCORES):
        o += results[c]["o"].astype(np.float64)
    return o.astype(np.float32)


def kernel(**inputs) -> np.ndarray:
    from concourse.bass_utils import run_bass_kernel_spmd

    nc, in_maps = _prepare(inputs)
    r = run_bass_kernel_spmd(nc, in_maps, list(range(NCORES)))
    return _reduce(r.results)
